# revision 14
# baseline (speedup 1.0000x reference)
"""ConvDeepSet Bass kernel for Trainium2 (8 NeuronCores, data-parallel over batch).

Math per batch b (reference):
    d[n,m]   = (x[n] - t[m])^2
    wt[n,m,c]= exp(-0.5 * d[n,m] / exp(sigma[c])^2) = exp(-alpha_c * d[n,m])
    ycat     = [ones, y]                       # (N, 9)
    yout[m,c]= sum_n ycat[n,c] * wt[n,m,c]     # (M, 9)
    h        = [yout[:,0], yout[:,1:]/(yout[:,0:1]+1e-8)]
    out      = h @ W + b                       # (M, 64)

Device mapping (one batch per core, n on partitions):
  Stage A+B ('derf' mode): one ACT pass per n-tile computes the Gaussian
           directly via Derivative_Erf(z) = (2/sqrt(pi)) * exp(-z^2) with
           z = sqrt(alpha)*t - sqrt(alpha)*x (scale/bias of the activation);
           the sqrt(pi)/2 constant is folded into ycat host-side. The
           n-contraction is fp32r matmuls with ycat as lhsT accumulating
           y_out[9, M] in PSUM (channels permuted, density last).
  Stage C: m-tile jj = {m : m = 16p + jj} (strided so each partition's 16
           output rows are contiguous in DRAM); matmul y_out slices against
           W_ext [9, 65] (col 64 selects the density row) into packed PSUM,
           then a batched DVE epilogue out = A*(1/(D+eps)) + D*W0 + bias and
           one output DMA with 4KB/partition contiguous runs.

All small operands arrive pre-broadcast/packed in two host-prepared arrays
(one fp32, one fp32r) to minimize DMA count. sigma is read on the host;
channels are grouped by unique alpha (the reference setup always produces a
single group, the fast path). Compiled programs are cached per grouping.
"""

import contextlib

import numpy as np

import concourse.bass as bass
import concourse.mybir as mybir
import concourse.tile as tile
from concourse import bacc
from concourse.bass_utils import run_bass_kernel_spmd

B, N, M = 8, 512, 2048
CIN = 8
C = CIN + 1          # 9 channels incl. density
O = 64
P = 128
NT = N // P          # 4 n-tiles
MT = M // P          # 16 m-tiles for stage C
F32 = mybir.dt.float32
F32R = mybir.dt.float32r
AF = mybir.ActivationFunctionType

_PROG_CACHE: dict = {}


def _aux_layout(G):
    """Column offsets in the packed fp32 aux array: negx | w0b | bb | wext.
    The wext region holds G blocks of (O+1) columns, each block's rows at
    partitions [0:ng] (base partition 0 for every group's matmul rhs)."""
    negx_ofs = 0
    w0_ofs = negx_ofs + G * NT
    bb_ofs = w0_ofs + O
    wext_ofs = bb_ofs + O
    total = wext_ofs + G * (O + 1)
    return negx_ofs, w0_ofs, bb_ofs, wext_ofs, total


def _build(group_sizes: tuple[int, ...], d_mode: str, alphas: tuple[float, ...],
           repeats: int | None = None):
    """group_sizes[g] channels share alphas[g]; density channel is the last
    channel of the last group. d_mode: 'derf' = single Derivative_Erf pass;
    'act' = Square pass + Exp pass (fallback). repeats wraps the compute body
    in a hardware loop (timing builds only)."""
    G = len(group_sizes)
    assert sum(group_sizes) == C
    negx_ofs, w0_ofs, bb_ofs, wext_ofs, FA = _aux_layout(G)
    nc = bacc.Bacc("TRN2", target_bir_lowering=False, debug=False)

    t_d = nc.dram_tensor("t_full", [M], F32, kind="ExternalInput")
    auxf_d = nc.dram_tensor("auxf", [P, FA], F32, kind="ExternalInput")
    auxr_d = nc.dram_tensor("auxr", [P, NT * C], F32R, kind="ExternalInput")
    out_d = nc.dram_tensor("out", [M, O], F32, kind="ExternalOutput")

    with tile.TileContext(nc) as tc:
        with (
            tc.tile_pool(name="singles", bufs=1) as singles,
            tc.tile_pool(name="work", bufs=2) as work,
            tc.tile_pool(name="psum", bufs=1, space=bass.MemorySpace.PSUM) as pp,
        ):
            # ---------------- setup (4 input DMAs total) ----------------
            t_b = singles.tile([P, M], F32)
            t_ap = t_d.ap()
            half_ap = lambda h: bass.AP(
                tensor=t_ap.tensor, offset=t_ap.offset + h * (M // 2),
                ap=[[0, P], [1, M // 2]],
            )
            nc.sync.dma_start(t_b[:, 0 : M // 2], half_ap(0))
            nc.scalar.dma_start(t_b[:, M // 2 : M], half_ap(1))

            auxf = singles.tile([P, FA], F32)
            nc.gpsimd.dma_start(auxf, auxf_d.ap())
            ycat = singles.tile([P, NT, C], F32R)
            nc.gpsimd.dma_start(ycat, auxr_d.ap().rearrange("p (a c) -> p a c", c=C))

            negx = auxf[:, negx_ofs : negx_ofs + G * NT].rearrange(
                "p (g a) -> p g a", g=G
            )
            w0b = auxf[:, w0_ofs : w0_ofs + O].rearrange("p (u o) -> p u o", u=1)
            bb = auxf[:, bb_ofs : bb_ofs + O].rearrange("p (u o) -> p u o", u=1)
            wg_tiles = []
            for g, ng in enumerate(group_sizes):
                cofs = wext_ofs + g * (O + 1)
                wg_tiles.append(auxf[0:ng, cofs : cofs + O + 1])

            loop_cm = tc.For_i(0, repeats, 1) if repeats else contextlib.nullcontext()
            with loop_cm:
                _bodyfn(nc, group_sizes, d_mode, alphas, G,
                        t_b, negx, ycat, wg_tiles, w0b, bb, out_d, work, pp)

    nc.compile()
    return nc


def _bodyfn(nc, group_sizes, d_mode, alphas, G,
            t_b, negx, ycat, wg_tiles, w0b, bb, out_d, work, pp):
    # ---------------- stages A+B ----------------
    yo_tiles = []
    if G == 1:
        psB = pp.tile([C, M], F32, tag="big")
    for g, ng in enumerate(group_sizes):
        if G > 1:
            psB = pp.tile([ng, M], F32, tag="big")
        sq_alpha = float(np.sqrt(alphas[g]))
        gofs = sum(group_sizes[:g])
        for i in range(NT):
            e = work.tile([P, M], F32R, tag="e")
            if d_mode == "derf":
                nc.scalar.activation(
                    e, t_b, AF.Derivative_Erf,
                    bias=negx[:, g, i : i + 1], scale=sq_alpha,
                )
            else:
                d_sb = work.tile([P, M], F32, tag="dsb")
                nc.scalar.activation(
                    d_sb, t_b, AF.Square,
                    bias=negx[:, g, i : i + 1], scale=sq_alpha,
                )
                nc.scalar.activation(e, d_sb, AF.Exp, scale=-1.0)
            lhsT = ycat[:, i, gofs : gofs + ng] if G > 1 else ycat[:, i, :]
            for j in range(M // 512):
                csl = slice(j * 512, (j + 1) * 512)
                nc.tensor.matmul(
                    psB[0:ng, csl] if G > 1 else psB[:, csl],
                    lhsT,
                    e[:, csl],
                    start=(i == 0),
                    stop=(i == NT - 1),
                )
        if G > 1:
            yo = work.tile([ng, M], F32, tag=f"yo{g}")
            nc.vector.tensor_copy(yo, psB)
            yo_tiles.append(yo)
    if G == 1:
        yo = work.tile([C, M], F32, tag="yo0")
        nc.vector.tensor_copy(yo, psB)
        yo_tiles.append(yo)

    # ---------------- stage C ----------------
    # m-tile jj covers m = 16p + jj (p = partition). Packed PSUM: tile jj at
    # cols [jj*128, jj*128+65) of a [P, 2048] fp32 region; 4 tiles per 2KB
    # bank; one zero region per bank (start only on the bank's first matmul).
    psC = pp.tile([P, MT, P], F32, tag="big")
    yo_str = [yo.rearrange("c (p j) -> c j p", j=MT) for yo in yo_tiles]
    for jj in range(MT):
        for g, ng in enumerate(group_sizes):
            first = (jj % 4 == 0) and (g == 0)
            nc.tensor.matmul(
                psC[:, jj, 0 : O + 1],
                yo_str[g][:, jj, :],
                wg_tiles[g],
                start=first,
                stop=(jj % 4 == 3) and (g == G - 1),
                skip_group_check=not first,
            )

    r = work.tile([P, MT, 1], F32, tag="r")
    nc.vector.tensor_scalar_add(r, psC[:, :, O : O + 1], 1e-8)
    nc.vector.reciprocal(r, r)
    osb = work.tile([P, MT, O], F32, tag="osb")
    # osb = A * r
    nc.vector.tensor_mul(osb, psC[:, :, 0:O], r.to_broadcast((P, MT, O)))
    # tmp = D * W0 ; osb += tmp ; osb += bias
    tmp = work.tile([P, MT, O], F32, tag="tmp")
    nc.vector.tensor_mul(
        tmp,
        psC[:, :, O : O + 1].to_broadcast((P, MT, O)),
        w0b.to_broadcast((P, MT, O)),
    )
    nc.vector.tensor_add(osb, osb, tmp)
    nc.vector.tensor_add(osb, osb, bb.to_broadcast((P, MT, O)))

    nc.sync.dma_start(out_d.ap().rearrange("(p j) o -> p j o", j=MT), osb)


def _get_prog(group_sizes, alphas, d_mode):
    key = (tuple(group_sizes), tuple(np.float32(a) for a in alphas), d_mode)
    if key not in _PROG_CACHE:
        _PROG_CACHE[key] = _build(tuple(group_sizes), d_mode, tuple(alphas))
    return _PROG_CACHE[key]


D_MODE = "derf"
SQRT_PI_2 = float(np.sqrt(np.pi) / 2.0)


def _host_prep(x, y, t, sigma, W, b):
    """Returns (group_sizes, alphas, in_maps)."""
    scales = np.exp(sigma.astype(np.float64))
    alphas_all = 0.5 / (scales * scales)          # (9,)

    # group channels by identical alpha; density channel (0) goes last
    uniq = []
    for cidx in range(C):
        a = np.float32(alphas_all[cidx])
        for gu in uniq:
            if gu[0] == a:
                gu[1].append(cidx)
                break
        else:
            uniq.append([a, [cidx]])
    gi = next(i for i, gu in enumerate(uniq) if 0 in gu[1])
    uniq.append(uniq.pop(gi))
    uniq[-1][1].remove(0)
    uniq[-1][1].append(0)
    perm = [cidx for _, chans in uniq for cidx in chans]       # length 9
    group_sizes = tuple(len(chans) for _, chans in uniq)
    alphas = tuple(float(a) for a, _ in uniq)
    G = len(group_sizes)

    W_perm = W[perm, :]                                        # (9, 64)
    wext = np.zeros((C, O + 1), np.float32)
    wext[:CIN, :O] = W_perm[:CIN, :]
    wext[CIN, O] = 1.0                                         # density selector
    yperm_cols = [cidx - 1 for cidx in perm if cidx != 0]      # 8 y columns

    ysc = SQRT_PI_2 if D_MODE == "derf" else 1.0
    sq_alphas = np.sqrt(np.array(alphas, np.float64))
    negx_ofs, w0_ofs, bb_ofs, wext_ofs, FA = _aux_layout(G)

    in_maps = []
    for bi in range(B):
        auxf = np.zeros((P, FA), np.float32)
        # negx: [p, g, a] with n = a*128 + p
        negx = (-sq_alphas[:, None] * x[bi][None, :]).astype(np.float32)  # (G, N)
        auxf[:, negx_ofs : negx_ofs + G * NT] = (
            negx.reshape(G, NT, P).transpose(2, 0, 1).reshape(P, G * NT)
        )
        auxf[:, w0_ofs : w0_ofs + O] = W[0, :][None, :]
        auxf[:, bb_ofs : bb_ofs + O] = b[None, :]
        gofs = 0
        for g, ng in enumerate(group_sizes):
            cofs = wext_ofs + g * (O + 1)
            auxf[:ng, cofs : cofs + O + 1] = wext[gofs : gofs + ng, :]
            gofs += ng

        ycat_host = np.concatenate(
            [y[bi][:, yperm_cols], np.ones((N, 1), np.float32)], axis=1
        ) * np.float32(ysc)                                    # (N, 9)
        auxr = ycat_host.reshape(NT, P, C).transpose(1, 0, 2).reshape(P, NT * C)

        in_maps.append({
            "t_full": np.ascontiguousarray(t[bi], np.float32),
            "auxf": np.ascontiguousarray(auxf),
            "auxr": np.ascontiguousarray(auxr, np.float32),
        })
    return group_sizes, alphas, in_maps


def kernel(x, y, t, sigma, W, b):
    x = np.asarray(x, np.float32).reshape(B, N)
    y = np.asarray(y, np.float32).reshape(B, N, CIN)
    t = np.asarray(t, np.float32).reshape(B, M)
    sigma = np.asarray(sigma, np.float32).reshape(C)
    W = np.asarray(W, np.float32).reshape(C, O)
    b = np.asarray(b, np.float32).reshape(O)

    group_sizes, alphas, in_maps = _host_prep(x, y, t, sigma, W, b)
    nc = _get_prog(group_sizes, alphas, D_MODE)

    res = run_bass_kernel_spmd(nc, in_maps, core_ids=list(range(B)))
    return np.stack([res.results[bi]["out"] for bi in range(B)], axis=0)


# revision 20
# speedup vs baseline: 2.0459x; 2.0459x over previous
"""ConvDeepSet Bass kernel for Trainium2 (8 NeuronCores, data-parallel over batch).

Math per batch b (reference):
    d[n,m]   = (x[n] - t[m])^2
    wt[n,m,c]= exp(-0.5 * d[n,m] / exp(sigma[c])^2) = exp(-alpha_c * d[n,m])
    ycat     = [ones, y]                       # (N, 9)
    yout[m,c]= sum_n ycat[n,c] * wt[n,m,c]     # (M, 9)
    h        = [yout[:,0], yout[:,1:]/(yout[:,0:1]+1e-8)]
    out      = h @ W + b                       # (M, 64)

Device mapping (one batch per core, n on partitions):
  Stage A+B ('derf' mode): one ACT pass per n-tile computes the Gaussian
           directly via Derivative_Erf(z) = (2/sqrt(pi)) * exp(-z^2) with
           z = sqrt(alpha)*t - sqrt(alpha)*x (scale/bias of the activation);
           the sqrt(pi)/2 constant is folded into ycat host-side. The
           n-contraction is fp32r matmuls with ycat as lhsT accumulating
           y_out[9, M] in PSUM (channels permuted, density last).
  Stage C: m-tile jj = {m : m = 16p + jj} (strided so each partition's 16
           output rows are contiguous in DRAM); matmul y_out slices against
           W_ext [9, 65] (col 64 selects the density row) into packed PSUM,
           then a batched DVE epilogue out = A*(1/(D+eps)) + D*W0 + bias and
           one output DMA with 4KB/partition contiguous runs.

All small operands arrive pre-broadcast/packed in two host-prepared arrays
(one fp32, one fp32r) to minimize DMA count. sigma is read on the host;
channels are grouped by unique alpha (the reference setup always produces a
single group, the fast path). Compiled programs are cached per grouping.
"""

import contextlib

import numpy as np

import concourse.bass as bass
import concourse.mybir as mybir
import concourse.tile as tile
from concourse import bacc
from concourse.bass_utils import run_bass_kernel_spmd

B, N, M = 8, 512, 2048
CIN = 8
C = CIN + 1          # 9 channels incl. density
O = 64
P = 128
NT = N // P          # 4 n-tiles
MT = M // P          # 16 m-tiles for stage C
F32 = mybir.dt.float32
F32R = mybir.dt.float32r
AF = mybir.ActivationFunctionType

_PROG_CACHE: dict = {}


def _aux_layout(G):
    """Column offsets in the packed fp32 aux array: negx | w0b | bb | wext.
    The wext region holds G blocks of (O+1) columns, each block's rows at
    partitions [0:ng] (base partition 0 for every group's matmul rhs)."""
    negx_ofs = 0
    w0_ofs = negx_ofs + G * NT
    bb_ofs = w0_ofs + O
    wext_ofs = bb_ofs + O
    total = wext_ofs + G * (O + 1)
    return negx_ofs, w0_ofs, bb_ofs, wext_ofs, total


def _build(group_sizes: tuple[int, ...], d_mode: str, alphas: tuple[float, ...],
           repeats: int | None = None):
    """group_sizes[g] channels share alphas[g]; density channel is the last
    channel of the last group. d_mode: 'derf' = single Derivative_Erf pass;
    'act' = Square pass + Exp pass (fallback). repeats wraps the compute body
    in a hardware loop (timing builds only)."""
    G = len(group_sizes)
    assert sum(group_sizes) == C
    negx_ofs, w0_ofs, bb_ofs, wext_ofs, FA = _aux_layout(G)
    nc = bacc.Bacc("TRN2", target_bir_lowering=False, debug=False)

    t_d = nc.dram_tensor("t_full", [M], F32, kind="ExternalInput")
    auxf_d = nc.dram_tensor("auxf", [P, FA], F32, kind="ExternalInput")
    auxr_d = nc.dram_tensor("auxr", [P, NT * C], F32R, kind="ExternalInput")
    out_d = nc.dram_tensor("out", [M, O], F32, kind="ExternalOutput")

    with tile.TileContext(nc) as tc:
        with (
            tc.tile_pool(name="singles", bufs=1) as singles,
            tc.tile_pool(name="work", bufs=2) as work,
            tc.tile_pool(name="psum", bufs=1, space=bass.MemorySpace.PSUM) as pp,
            tc.tile_pool(name="psum2", bufs=2, space=bass.MemorySpace.PSUM) as pp2,
        ):
            # ---------------- setup (4 input DMAs total) ----------------
            t_b = singles.tile([P, M], F32)
            t_ap = t_d.ap()
            half_ap = lambda h: bass.AP(
                tensor=t_ap.tensor, offset=t_ap.offset + h * (M // 2),
                ap=[[0, P], [1, M // 2]],
            )
            nc.sync.dma_start(t_b[:, 0 : M // 2], half_ap(0))
            nc.scalar.dma_start(t_b[:, M // 2 : M], half_ap(1))

            auxf = singles.tile([P, FA], F32)
            nc.gpsimd.dma_start(auxf, auxf_d.ap())
            ycat = singles.tile([P, NT, C], F32R)
            nc.gpsimd.dma_start(ycat, auxr_d.ap().rearrange("p (a c) -> p a c", c=C))
            # (negx arrives with auxf before the first ACT pass; ycat only
            # gates the first matmul)

            negx = auxf[:, negx_ofs : negx_ofs + G * NT].rearrange(
                "p (g a) -> p g a", g=G
            )
            w0b = auxf[:, w0_ofs : w0_ofs + O].rearrange("p (u o) -> p u o", u=1)
            bb = auxf[:, bb_ofs : bb_ofs + O].rearrange("p (u o) -> p u o", u=1)
            wg_tiles = []
            for g, ng in enumerate(group_sizes):
                cofs = wext_ofs + g * (O + 1)
                wg_tiles.append(auxf[0:ng, cofs : cofs + O + 1])

            loop_cm = tc.For_i(0, repeats, 1) if repeats else contextlib.nullcontext()
            with loop_cm:
                _bodyfn(nc, group_sizes, d_mode, alphas, G,
                        t_b, negx, ycat, wg_tiles, w0b, bb, out_d, work, pp, pp2)

    nc.compile()
    return nc


def _bodyfn(nc, group_sizes, d_mode, alphas, G,
            t_b, negx, ycat, wg_tiles, w0b, bb, out_d, work, pp, pp2):
    MH = M // 2          # m-half width
    JT = MH // P         # 8 m-tiles per half
    # ---------------- stages A+B ----------------
    yo_tiles = []        # [g][h] -> yo half tile
    if G == 1:
        psB = pp.tile([C, M], F32, tag="big")
    for g, ng in enumerate(group_sizes):
        if G > 1:
            psB = pp.tile([ng, M], F32, tag="big")
        sq_alpha = float(np.sqrt(alphas[g]))
        gofs = sum(group_sizes[:g])
        for i in range(NT):
            e = work.tile([P, M], F32R, tag="e")
            if d_mode == "derf":
                nc.scalar.activation(
                    e, t_b, AF.Derivative_Erf,
                    bias=negx[:, g, i : i + 1], scale=sq_alpha,
                )
            else:
                d_sb = work.tile([P, M], F32, tag="dsb")
                nc.scalar.activation(
                    d_sb, t_b, AF.Square,
                    bias=negx[:, g, i : i + 1], scale=sq_alpha,
                )
                nc.scalar.activation(e, d_sb, AF.Exp, scale=-1.0)
            lhsT = ycat[:, i, gofs : gofs + ng] if G > 1 else ycat[:, i, :]
            for j in range(M // 512):
                csl = slice(j * 512, (j + 1) * 512)
                nc.tensor.matmul(
                    psB[0:ng, csl] if G > 1 else psB[:, csl],
                    lhsT,
                    e[:, csl],
                    start=(i == 0),
                    stop=(i == NT - 1),
                )
        yo_halves = []
        for h in range(2):
            yo = work.tile([ng if G > 1 else C, MH], F32, tag=f"yo{g}h{h}")
            nc.vector.tensor_copy(yo, psB[0 : (ng if G > 1 else C),
                                          h * MH : (h + 1) * MH])
            yo_halves.append(yo)
        yo_tiles.append(yo_halves)

    # ---------------- stage C (pipelined per m-half) ----------------
    # Within half h, m-tile jj covers global m = h*MH + 8p + jj, so each
    # partition's 8 output rows are DRAM-contiguous (2KB runs). Packed PSUM:
    # tile jj at cols [jj*128, jj*128+65) of a [P, MH] fp32 region (2 banks);
    # one zero region per 4 tiles (start only on the bank's first matmul).
    for h in range(2):
        psC = pp2.tile([P, JT, P], F32, tag="psc")
        yo_str = [yo_tiles[g][h].rearrange("c (p j) -> c j p", j=JT)
                  for g in range(G)]
        for jj in range(JT):
            for g, ng in enumerate(group_sizes):
                first = (jj % 4 == 0) and (g == 0)
                nc.tensor.matmul(
                    psC[:, jj, 0 : O + 1],
                    yo_str[g][:, jj, :],
                    wg_tiles[g],
                    start=first,
                    stop=(jj % 4 == 3) and (g == G - 1),
                    skip_group_check=not first,
                )

        r = work.tile([P, JT, 1], F32, tag="r")
        nc.vector.tensor_scalar_add(r, psC[:, :, O : O + 1], 1e-8)
        nc.vector.reciprocal(r, r)
        osb = work.tile([P, JT, O], F32, tag="osb")
        # osb = A * r
        nc.vector.tensor_mul(osb, psC[:, :, 0:O], r.to_broadcast((P, JT, O)))
        # tmp = D * W0 ; osb += tmp ; osb += bias
        tmp = work.tile([P, JT, O], F32, tag="tmp")
        nc.vector.tensor_mul(
            tmp,
            psC[:, :, O : O + 1].to_broadcast((P, JT, O)),
            w0b.to_broadcast((P, JT, O)),
        )
        nc.vector.tensor_add(osb, osb, tmp)
        nc.vector.tensor_add(osb, osb, bb.to_broadcast((P, JT, O)))

        out_ap = out_d.ap()[h * MH : (h + 1) * MH, :].rearrange(
            "(p j) o -> p j o", j=JT
        )
        (nc.sync if h == 0 else nc.gpsimd).dma_start(out_ap, osb)


def _get_prog(group_sizes, alphas, d_mode):
    key = (tuple(group_sizes), tuple(np.float32(a) for a in alphas), d_mode)
    if key not in _PROG_CACHE:
        _PROG_CACHE[key] = _build(tuple(group_sizes), d_mode, tuple(alphas))
    return _PROG_CACHE[key]


D_MODE = "derf"
SQRT_PI_2 = float(np.sqrt(np.pi) / 2.0)


def _host_prep(x, y, t, sigma, W, b):
    """Returns (group_sizes, alphas, in_maps)."""
    scales = np.exp(sigma.astype(np.float64))
    alphas_all = 0.5 / (scales * scales)          # (9,)

    # group channels by identical alpha; density channel (0) goes last
    uniq = []
    for cidx in range(C):
        a = np.float32(alphas_all[cidx])
        for gu in uniq:
            if gu[0] == a:
                gu[1].append(cidx)
                break
        else:
            uniq.append([a, [cidx]])
    gi = next(i for i, gu in enumerate(uniq) if 0 in gu[1])
    uniq.append(uniq.pop(gi))
    uniq[-1][1].remove(0)
    uniq[-1][1].append(0)
    perm = [cidx for _, chans in uniq for cidx in chans]       # length 9
    group_sizes = tuple(len(chans) for _, chans in uniq)
    alphas = tuple(float(a) for a, _ in uniq)
    G = len(group_sizes)

    W_perm = W[perm, :]                                        # (9, 64)
    wext = np.zeros((C, O + 1), np.float32)
    wext[:CIN, :O] = W_perm[:CIN, :]
    wext[CIN, O] = 1.0                                         # density selector
    yperm_cols = [cidx - 1 for cidx in perm if cidx != 0]      # 8 y columns

    ysc = SQRT_PI_2 if D_MODE == "derf" else 1.0
    sq_alphas = np.sqrt(np.array(alphas, np.float64))
    negx_ofs, w0_ofs, bb_ofs, wext_ofs, FA = _aux_layout(G)

    in_maps = []
    for bi in range(B):
        auxf = np.zeros((P, FA), np.float32)
        # negx: [p, g, a] with n = a*128 + p
        negx = (-sq_alphas[:, None] * x[bi][None, :]).astype(np.float32)  # (G, N)
        auxf[:, negx_ofs : negx_ofs + G * NT] = (
            negx.reshape(G, NT, P).transpose(2, 0, 1).reshape(P, G * NT)
        )
        auxf[:, w0_ofs : w0_ofs + O] = W[0, :][None, :]
        auxf[:, bb_ofs : bb_ofs + O] = b[None, :]
        gofs = 0
        for g, ng in enumerate(group_sizes):
            cofs = wext_ofs + g * (O + 1)
            auxf[:ng, cofs : cofs + O + 1] = wext[gofs : gofs + ng, :]
            gofs += ng

        ycat_host = np.concatenate(
            [y[bi][:, yperm_cols], np.ones((N, 1), np.float32)], axis=1
        ) * np.float32(ysc)                                    # (N, 9)
        auxr = ycat_host.reshape(NT, P, C).transpose(1, 0, 2).reshape(P, NT * C)

        in_maps.append({
            "t_full": np.ascontiguousarray(t[bi], np.float32),
            "auxf": np.ascontiguousarray(auxf),
            "auxr": np.ascontiguousarray(auxr, np.float32),
        })
    return group_sizes, alphas, in_maps


def kernel(x, y, t, sigma, W, b):
    x = np.asarray(x, np.float32).reshape(B, N)
    y = np.asarray(y, np.float32).reshape(B, N, CIN)
    t = np.asarray(t, np.float32).reshape(B, M)
    sigma = np.asarray(sigma, np.float32).reshape(C)
    W = np.asarray(W, np.float32).reshape(C, O)
    b = np.asarray(b, np.float32).reshape(O)

    group_sizes, alphas, in_maps = _host_prep(x, y, t, sigma, W, b)
    nc = _get_prog(group_sizes, alphas, D_MODE)

    res = run_bass_kernel_spmd(nc, in_maps, core_ids=list(range(B)))
    return np.stack([res.results[bi]["out"] for bi in range(B)], axis=0)
